# revision 35
# baseline (speedup 1.0000x reference)
"""Class-based decoder (MoE-style routing) on 8 trn2 NeuronCores.

Expert-parallel: classes padded 250->256, 32 per core.  Tokens are grouped
by class into capacity-16 slots on the host (rare overflow tokens beyond a
class's 16-slot capacity are evaluated directly on the host in numpy).
Each core owns 32 class slots = 512 padded token rows = 4 m-tiles of 128.

Memory-bound problem: the dominant HBM traffic is the per-class word-decoder
stack (250*200*512).  It is streamed as fp8 e3m4 (4 mantissa bits, ~1.8% rms
quantization error on the word logits only -> ~0.8% overall rel err, inside
the 2e-2 gate), halving W bytes vs bf16.  x / Wc stay bf16.  A global
power-of-2 scale (W*128, x/128) keeps the fp8 values in e3m4's normal range
and is exactly undone in the product.

Per 128-token m-tile:
  - class logits: 4 K-chunk matmuls, full 128x128 array, N=256.
  - word logits: classes are paired per 32-row band; for each of the 4 bands
    a M=32 col-tiled matmul (tile_position=(0,32h)) accumulates K=512 into a
    shared PSUM tile [128, 400].  Col tiles run concurrently on the PE.
  - m-tiles 0-2: the per-band diagonal (each 16-row half keeps its own
    class's 200 of the 400 pair columns) is selected on-chip:
        t1 = a*mask (DVE), t2 = b*(1-mask) (ACT scaled copy),
        words = t1 + t2 (DVE, bf16 out) -> store [128, 450].
  - m-tile 3 (the critical tail): stored WIDE [128, 650] with one parallel
    DVE(class)+ACT(pair block) copy - a one-hop evacuation; the host picks
    the diagonal for these rows.

DMA: the W stream rides the sync (SP HWDGE) ring as seven 0.41MB chunks
followed by two single-pair 0.2MB chunks, so the last chunk's completion
semaphore (which gates the final matmuls) fires with the fabric quiet.
x/wc/mask load on the scalar ring; m-tile 0-2 stores interleave with the
stream on the gpsimd (SWDGE) queue; the final wide store rides the
then-idle scalar ring.  The whole W shard (3.28MB fp8) is SBUF-resident.
"""

import numpy as np
from contextlib import ExitStack

import concourse.bass as bass
import concourse.bacc as bacc
import concourse.tile as tile
import concourse.mybir as mybir
from concourse.bass_utils import run_bass_kernel_spmd

import ml_dtypes

NHID = 512
NCLS = 250
CHUNK = 200
NCORES = 8
KCH = NHID // 128           # 4 contraction chunks of 128
NCLS_PAD = 256
CPC = NCLS_PAD // NCORES    # 32 classes per core
C = 16                      # token capacity per class slot
SLOTS = CPC                 # one slot per owned class
NPAD = SLOTS * C            # 512 padded token rows per core
N_MT = NPAD // 128          # 4 m-tiles
PAIRS_MT = 128 // (2 * C)   # 4 class-pairs (32-row bands) per m-tile
NPAIRS = N_MT * PAIRS_MT    # 16 pairs per core
NPAIR = 2 * CHUNK           # 400 word columns per pair matmul
PCH = KCH * NPAIR           # 1600 W columns per pair in the flat layout
NCOL = NCLS + CHUNK         # 450 output columns
WCOL = NCLS + NPAIR         # 650 wide output columns (last m-tile)
# W DMA chunks in pairs: big chunks first, two single-pair chunks last so
# the final completion semaphore fires on a quiet fabric
WCHUNKS = [2, 2, 2, 2, 2, 2, 2, 1, 1]
SCL = 128.0                 # fp8 pre-scale: W*SCL, x/SCL

F32 = mybir.dt.float32
BF16 = mybir.dt.bfloat16
FP8 = mybir.dt.float8e3
NP_BF16 = ml_dtypes.bfloat16
NP_FP8 = ml_dtypes.float8_e3m4

LAST_RESULT = None
_program_cache = {}


def _build_program():
    nc = bacc.Bacc("TRN2", target_bir_lowering=False, debug=False,
                   num_devices=NCORES)
    xT = nc.dram_tensor("xT", [128, N_MT * KCH * 128], BF16,
                        kind="ExternalInput")
    wcT = nc.dram_tensor("wcT", [128, KCH * NCLS_PAD], BF16,
                         kind="ExternalInput")
    wwT = nc.dram_tensor("wwT", [128, NPAIRS * PCH], FP8,
                         kind="ExternalInput")
    msk = nc.dram_tensor("msk", [128, 8], F32, kind="ExternalInput")
    out = nc.dram_tensor("out", [(N_MT - 1) * 128, NCOL], BF16,
                         kind="ExternalOutput")
    out2 = nc.dram_tensor("out2", [128, WCOL], BF16, kind="ExternalOutput")

    with tile.TileContext(nc) as tc, ExitStack() as ctx:
        xpool = ctx.enter_context(tc.tile_pool(name="x", bufs=1))
        wcpool = ctx.enter_context(tc.tile_pool(name="wc", bufs=1))
        mpool = ctx.enter_context(tc.tile_pool(name="m", bufs=1))
        wpool = ctx.enter_context(tc.tile_pool(name="w", bufs=1))
        dpool = ctx.enter_context(tc.tile_pool(name="d", bufs=3))
        opool = ctx.enter_context(tc.tile_pool(name="o", bufs=4))
        pcp = ctx.enter_context(
            tc.tile_pool(name="pc", bufs=1, space=bass.MemorySpace.PSUM))
        pwp = ctx.enter_context(
            tc.tile_pool(name="pw", bufs=1, space=bass.MemorySpace.PSUM))

        x_sb = xpool.tile([128, N_MT * KCH * 128], BF16)
        wc_sb = wcpool.tile([128, KCH * NCLS_PAD], BF16)
        m_sb = mpool.tile([128, 8], F32)
        w_sb = wpool.tile([128, NPAIRS * PCH], FP8)

        # loads: x/wc/mask on the scalar ring, W chunks on the sync ring
        nc.scalar.dma_start(x_sb[:, 0:KCH * 128], xT[:, 0:KCH * 128])
        nc.scalar.dma_start(wc_sb[:], wcT[:])
        nc.scalar.dma_start(m_sb[:], msk[:])
        for m in range(1, N_MT):
            s = slice(m * KCH * 128, (m + 1) * KCH * 128)
            nc.scalar.dma_start(x_sb[:, s], xT[:, s])
        p0 = 0
        for npair in WCHUNKS:
            s = slice(p0 * PCH, (p0 + npair) * PCH)
            nc.sync.dma_start(w_sb[:, s], wwT[:, s])
            p0 += npair

        def xcol(m, j, lo, hi):
            base = (m * KCH + j) * 128
            return x_sb[:, base + lo:base + hi]

        # ---- HAM warm-up: ~4us of tiny dummy matmuls so the PE clock is
        # un-throttled (2.4GHz) by the time the real matmuls start; they
        # run while the first DMAs stream and cost no critical-path time.
        # The dummy target reuses the last pw PSUM bank (WAW-ordered). ----
        dm_sb = mpool.tile([128, 80], BF16, tag="dm")
        nc.vector.memset(dm_sb[:], 0.0)
        warm_ps = pwp.tile([128, NPAIR], F32, tag=f"pw{N_MT - 1}")
        for _ in range(52):
            nc.tensor.matmul(warm_ps[0:8, 0:64], dm_sb[:, 0:8],
                             dm_sb[:, 16:80], start=True, stop=True)

        # ---- class logits first (x/wc arrive early; the warm-up covers
        # the wait), so the word matmuls of the last m-tile are never
        # stuck behind them at the tail ----
        pc_ps = []
        for m in range(N_MT):
            ps = pcp.tile([128, NCLS_PAD], F32, tag=f"pc{m}", name=f"pc{m}")
            for j in range(KCH):
                nc.tensor.matmul(
                    ps[:, :], xcol(m, j, 0, 128),
                    wc_sb[:, j * NCLS_PAD:(j + 1) * NCLS_PAD],
                    start=(j == 0), stop=(j == KCH - 1))
            pc_ps.append(ps)

        for m in range(N_MT):
            # ---- word logits: col-tiled M=32 matmuls per 32-row band ----
            pw_ps = pwp.tile([128, NPAIR], F32, tag=f"pw{m}")
            for h in range(PAIRS_MT):
                p = m * PAIRS_MT + h
                for j in range(KCH):
                    nc.tensor.matmul(
                        pw_ps[32 * h:32 * (h + 1), :],
                        xcol(m, j, 32 * h, 32 * (h + 1)),
                        w_sb[:, (p * KCH + j) * NPAIR:
                             (p * KCH + j + 1) * NPAIR],
                        start=(j == 0), stop=(j == KCH - 1),
                        tile_position=(0, 32 * h))

            if m < N_MT - 1:
                # ---- select the per-band diagonal on-chip; store 450 ----
                o_sb = opool.tile([128, NCOL], BF16, tag="o")
                d_sb = dpool.tile([128, 2 * CHUNK], BF16, tag="d")
                nc.vector.tensor_copy(o_sb[:, :NCLS], pc_ps[m][:, :NCLS])
                nc.vector.tensor_scalar_mul(
                    d_sb[:, :CHUNK], pw_ps[:, :CHUNK], m_sb[:, 0:1])
                nc.scalar.activation(
                    d_sb[:, CHUNK:], pw_ps[:, CHUNK:],
                    mybir.ActivationFunctionType.Copy, scale=m_sb[:, 1:2])
                nc.vector.tensor_tensor(
                    o_sb[:, NCLS:], d_sb[:, :CHUNK], d_sb[:, CHUNK:],
                    mybir.AluOpType.add)
                nc.gpsimd.dma_start(out[m * 128:(m + 1) * 128, :], o_sb[:])
            else:
                # ---- last m-tile: one-hop wide evacuation, host selects —
                # DVE and ACT copy disjoint halves in parallel ----
                o2_sb = opool.tile([128, WCOL], BF16, tag="o2")
                nc.vector.tensor_copy(o2_sb[:, :NCLS], pc_ps[m][:, :NCLS])
                nc.scalar.activation(o2_sb[:, NCLS:], pw_ps[:, :],
                                     mybir.ActivationFunctionType.Copy)
                nc.scalar.dma_start(out2[:, :], o2_sb[:])

    nc.compile()
    return nc


def _route(cls):
    """Group tokens by class into capacity-C slots; tokens beyond capacity
    are returned as `overflow` and evaluated on the host in numpy."""
    counts = np.bincount(cls, minlength=NCLS_PAD)
    order = np.argsort(cls, kind="stable")
    starts = np.zeros(NCLS_PAD + 1, np.int64)
    starts[1:] = np.cumsum(counts)

    tok_idx = np.full((NCORES, NPAD), -1, np.int64)
    overflow = []
    for k in range(NCORES):
        for s in range(SLOTS):
            c = k * CPC + s
            lo, cnt = int(starts[c]), int(counts[c])
            n = min(C, cnt)
            if n > 0:
                tok_idx[k, s * C:s * C + n] = order[lo:lo + n]
            if cnt > C:
                overflow.append(order[lo + C:lo + cnt])
    overflow = (np.concatenate(overflow) if overflow
                else np.zeros((0,), np.int64))
    return tok_idx, overflow


def kernel(x, Wc, bc, Ww, bw, cls_idx, _trace=False, _trace_cores=None):
    global LAST_RESULT
    x = np.ascontiguousarray(np.asarray(x, np.float32))
    Wc = np.ascontiguousarray(np.asarray(Wc, np.float32))
    bc = np.asarray(bc, np.float32)
    Ww = np.ascontiguousarray(np.asarray(Ww, np.float32))
    bw = np.asarray(bw, np.float32)
    cls = np.asarray(cls_idx).astype(np.int64).ravel()
    N = cls.shape[0]

    tok_idx, overflow = _route(cls)

    if "prog" not in _program_cache:
        _program_cache["prog"] = _build_program()
    nc = _program_cache["prog"]

    # wcT [128, KCH*256]: wcT[p, j*256+c] = (Wc*SCL)[c, j*128+p], replicated
    Wc_p = np.zeros((NCLS_PAD, NHID), np.float32)
    Wc_p[:NCLS] = Wc * SCL
    wcT = np.ascontiguousarray(
        Wc_p.reshape(NCLS_PAD, KCH, 128).transpose(2, 1, 0)
            .reshape(128, KCH * NCLS_PAD).astype(NP_BF16))

    # per-partition 16-row parity masks: col0 = 1 on first-class rows of each
    # pair, col1 = complement
    msk_np = np.zeros((128, 8), np.float32)
    msk_np[:, 0] = 1.0 - ((np.arange(128) // C) % 2)
    msk_np[:, 1] = (np.arange(128) // C) % 2

    Ww_pad = np.zeros((NCLS_PAD, CHUNK, NHID), np.float32)
    Ww_pad[:NCLS] = Ww * SCL

    in_maps = []
    for k in range(NCORES):
        # flat pair-major layout:
        # wwT[p, (pair*KCH+j)*400 + a*200 + w] = Ww_pad[cls(slot), w, j*128+p]
        wsel = Ww_pad[np.arange(k * CPC, (k + 1) * CPC)]     # [32,200,512]
        tmp = wsel.reshape(SLOTS, CHUNK, KCH, 128).transpose(0, 2, 3, 1)
        tmp = tmp.reshape(NPAIRS, 2, KCH, 128, CHUNK)
        tmp = tmp.transpose(3, 0, 2, 1, 4)     # [128, pair, j, 2, CHUNK]
        wwT = np.ascontiguousarray(
            tmp.reshape(128, NPAIRS * PCH).astype(NP_FP8))

        ti = tok_idx[k]
        xk = x[np.maximum(ti, 0)] * (1.0 / SCL)
        xk[ti < 0] = 0.0
        xT = np.ascontiguousarray(
            xk.reshape(N_MT, 128, KCH, 128).transpose(3, 0, 2, 1)
              .reshape(128, N_MT * KCH * 128).astype(NP_BF16))
        in_maps.append({"xT": xT, "wcT": wcT, "wwT": wwT, "msk": msk_np})

    LAST_RESULT = run_bass_kernel_spmd(
        nc, in_maps, list(range(NCORES)), trace=_trace,
        trace_cores=(_trace_cores if _trace else None))

    out = np.zeros((N, NCOL), np.float32)
    # last m-tile rows come back wide; slot parity picks the 200-col half
    a_row = ((np.arange(128) // C) % 2)[:, None]
    for k in range(NCORES):
        res = LAST_RESULT.results[k]
        ok = np.asarray(res["out"]).astype(np.float32)
        o2 = np.asarray(res["out2"]).astype(np.float32)
        words2 = np.where(a_row == 0, o2[:, NCLS:NCLS + CHUNK],
                          o2[:, NCLS + CHUNK:])
        full = np.concatenate(
            [ok, np.concatenate([o2[:, :NCLS], words2], 1)], 0)
        valid = tok_idx[k] >= 0
        out[tok_idx[k][valid]] = full[valid]

    if overflow.size:
        xo = x[overflow]
        out[overflow, :NCLS] = xo @ Wc.T
        co = cls[overflow]
        out[overflow, NCLS:] = np.einsum(
            "nkh,nh->nk", Ww[co], xo, optimize=True)

    out[:, :NCLS] += bc
    out[:, NCLS:] += bw[cls]
    return out


# revision 36
# speedup vs baseline: 1.1104x; 1.1104x over previous
"""Class-based decoder (MoE-style routing) on 8 trn2 NeuronCores.

Expert-parallel: classes padded 250->256, 32 per core.  Tokens are grouped
by class into capacity-16 slots on the host (rare overflow tokens beyond a
class's 16-slot capacity are evaluated directly on the host in numpy).
Each core owns 32 class slots = 512 padded token rows = 4 m-tiles of 128.

Memory-bound problem: the dominant HBM traffic is the per-class word-decoder
stack (250*200*512).  It is streamed as fp8 e3m4 (4 mantissa bits, ~1.8% rms
quantization error on the word logits only -> ~0.8% overall rel err, inside
the 2e-2 gate), halving W bytes vs bf16.  x / Wc stay bf16.  A global
power-of-2 scale (W*128, x/128) keeps the fp8 values in e3m4's normal range
and is exactly undone in the product.

Per 128-token m-tile:
  - class logits: 4 K-chunk matmuls, full 128x128 array, N=256.
  - word logits: classes are paired per 32-row band; for each of the 4 bands
    a M=32 col-tiled matmul (tile_position=(0,32h)) accumulates K=512 into a
    shared PSUM tile [128, 400].  Col tiles run concurrently on the PE.
  - m-tiles 0-2: the per-band diagonal (each 16-row half keeps its own
    class's 200 of the 400 pair columns) is selected on-chip:
        t1 = a*mask (DVE), t2 = b*(1-mask) (ACT scaled copy),
        words = t1 + t2 (DVE, bf16 out) -> store [128, 450].
  - m-tile 3 (the critical tail): stored WIDE [128, 650] with one parallel
    DVE(class)+ACT(pair block) copy - a one-hop evacuation; the host picks
    the diagonal for these rows.

DMA: the W stream rides the sync (SP HWDGE) ring as seven 0.41MB chunks
followed by two single-pair 0.2MB chunks, so the last chunk's completion
semaphore (which gates the final matmuls) fires with the fabric quiet.
x/wc/mask load on the scalar ring; m-tile 0-2 stores interleave with the
stream on the gpsimd (SWDGE) queue; the final wide store rides the
then-idle scalar ring.  The whole W shard (3.28MB fp8) is SBUF-resident.
"""

import numpy as np
from contextlib import ExitStack

import concourse.bass as bass
import concourse.bacc as bacc
import concourse.tile as tile
import concourse.mybir as mybir
from concourse.bass_utils import run_bass_kernel_spmd

import ml_dtypes

NHID = 512
NCLS = 250
CHUNK = 200
NCORES = 8
KCH = NHID // 128           # 4 contraction chunks of 128
NCLS_PAD = 256
CPC = NCLS_PAD // NCORES    # 32 classes per core
C = 16                      # token capacity per class slot
SLOTS = CPC                 # one slot per owned class
NPAD = SLOTS * C            # 512 padded token rows per core
N_MT = NPAD // 128          # 4 m-tiles
PAIRS_MT = 128 // (2 * C)   # 4 class-pairs (32-row bands) per m-tile
NPAIRS = N_MT * PAIRS_MT    # 16 pairs per core
NPAIR = 2 * CHUNK           # 400 word columns per pair matmul
PCH = KCH * NPAIR           # 1600 W columns per pair in the flat layout
NCOL = NCLS + CHUNK         # 450 output columns
WCOL = NCLS + NPAIR         # 650 wide output columns (last m-tile)
# W DMA chunks in pairs: big chunks first, two single-pair chunks last so
# the final completion semaphore fires on a quiet fabric
WCHUNKS = [2, 2, 2, 2, 2, 2, 2, 1, 1]
SCL = 128.0                 # fp8 pre-scale: W*SCL, x/SCL

F32 = mybir.dt.float32
BF16 = mybir.dt.bfloat16
FP8 = mybir.dt.float8e3
NP_BF16 = ml_dtypes.bfloat16
NP_FP8 = ml_dtypes.float8_e3m4

LAST_RESULT = None
_program_cache = {}


def _build_program():
    nc = bacc.Bacc("TRN2", target_bir_lowering=False, debug=False,
                   num_devices=NCORES)
    xT = nc.dram_tensor("xT", [128, N_MT * KCH * 128], BF16,
                        kind="ExternalInput")
    wcT = nc.dram_tensor("wcT", [128, KCH * NCLS_PAD], BF16,
                         kind="ExternalInput")
    wwT = nc.dram_tensor("wwT", [128, NPAIRS * PCH], FP8,
                         kind="ExternalInput")
    msk = nc.dram_tensor("msk", [128, 8], F32, kind="ExternalInput")
    out = nc.dram_tensor("out", [(N_MT - 1) * 128, NCOL], BF16,
                         kind="ExternalOutput")
    out2 = nc.dram_tensor("out2", [128, WCOL], BF16, kind="ExternalOutput")

    with tile.TileContext(nc) as tc, ExitStack() as ctx:
        xpool = ctx.enter_context(tc.tile_pool(name="x", bufs=1))
        wcpool = ctx.enter_context(tc.tile_pool(name="wc", bufs=1))
        mpool = ctx.enter_context(tc.tile_pool(name="m", bufs=1))
        wpool = ctx.enter_context(tc.tile_pool(name="w", bufs=1))
        dpool = ctx.enter_context(tc.tile_pool(name="d", bufs=3))
        opool = ctx.enter_context(tc.tile_pool(name="o", bufs=4))
        pcp = ctx.enter_context(
            tc.tile_pool(name="pc", bufs=1, space=bass.MemorySpace.PSUM))
        pwp = ctx.enter_context(
            tc.tile_pool(name="pw", bufs=1, space=bass.MemorySpace.PSUM))

        x_sb = xpool.tile([128, N_MT * KCH * 128], BF16)
        wc_sb = wcpool.tile([128, KCH * NCLS_PAD], BF16)
        m_sb = mpool.tile([128, 8], F32)
        w_sb = wpool.tile([128, NPAIRS * PCH], FP8)

        # loads: x/wc/mask on the scalar ring, W chunks on the sync ring
        nc.scalar.dma_start(x_sb[:, 0:KCH * 128], xT[:, 0:KCH * 128])
        nc.scalar.dma_start(wc_sb[:], wcT[:])
        nc.scalar.dma_start(m_sb[:], msk[:])
        for m in range(1, N_MT):
            s = slice(m * KCH * 128, (m + 1) * KCH * 128)
            nc.scalar.dma_start(x_sb[:, s], xT[:, s])
        p0 = 0
        for npair in WCHUNKS:
            s = slice(p0 * PCH, (p0 + npair) * PCH)
            nc.sync.dma_start(w_sb[:, s], wwT[:, s])
            p0 += npair

        def xcol(m, j, lo, hi):
            base = (m * KCH + j) * 128
            return x_sb[:, base + lo:base + hi]

        # ---- HAM warm-up: ~4us of tiny dummy matmuls so the PE clock is
        # un-throttled (2.4GHz) by the time the real matmuls start; they
        # run while the first DMAs stream and cost no critical-path time.
        # The dummy target reuses the last pw PSUM bank (WAW-ordered). ----
        dm_sb = mpool.tile([128, 80], BF16, tag="dm")
        nc.vector.memset(dm_sb[:], 0.0)
        warm_ps = pwp.tile([128, NPAIR], F32, tag=f"pw{N_MT - 1}")
        for _ in range(36):
            nc.tensor.matmul(warm_ps[0:8, 0:64], dm_sb[:, 0:8],
                             dm_sb[:, 16:80], start=True, stop=True)

        # class-logit matmuls: interleaved with the word matmuls of the
        # previous m-tile, except pc3 which is hoisted before pw2 so the
        # last m-tile's word matmuls have nothing in front of them
        pc_ps = [None] * N_MT

        def pc_mms(m):
            ps = pcp.tile([128, NCLS_PAD], F32, tag=f"pc{m}", name=f"pc{m}")
            for j in range(KCH):
                nc.tensor.matmul(
                    ps[:, :], xcol(m, j, 0, 128),
                    wc_sb[:, j * NCLS_PAD:(j + 1) * NCLS_PAD],
                    start=(j == 0), stop=(j == KCH - 1))
            pc_ps[m] = ps

        for m in range(N_MT):
            if pc_ps[m] is None:
                pc_mms(m)
            if m == N_MT - 2:
                pc_mms(N_MT - 1)

            # ---- word logits: col-tiled M=32 matmuls per 32-row band ----
            pw_ps = pwp.tile([128, NPAIR], F32, tag=f"pw{m}")
            for h in range(PAIRS_MT):
                p = m * PAIRS_MT + h
                for j in range(KCH):
                    nc.tensor.matmul(
                        pw_ps[32 * h:32 * (h + 1), :],
                        xcol(m, j, 32 * h, 32 * (h + 1)),
                        w_sb[:, (p * KCH + j) * NPAIR:
                             (p * KCH + j + 1) * NPAIR],
                        start=(j == 0), stop=(j == KCH - 1),
                        tile_position=(0, 32 * h))

            if m < N_MT - 1:
                # ---- select the per-band diagonal on-chip; store 450 ----
                o_sb = opool.tile([128, NCOL], BF16, tag="o")
                d_sb = dpool.tile([128, 2 * CHUNK], BF16, tag="d")
                nc.vector.tensor_copy(o_sb[:, :NCLS], pc_ps[m][:, :NCLS])
                nc.vector.tensor_scalar_mul(
                    d_sb[:, :CHUNK], pw_ps[:, :CHUNK], m_sb[:, 0:1])
                nc.scalar.activation(
                    d_sb[:, CHUNK:], pw_ps[:, CHUNK:],
                    mybir.ActivationFunctionType.Copy, scale=m_sb[:, 1:2])
                nc.vector.tensor_tensor(
                    o_sb[:, NCLS:], d_sb[:, :CHUNK], d_sb[:, CHUNK:],
                    mybir.AluOpType.add)
                nc.gpsimd.dma_start(out[m * 128:(m + 1) * 128, :], o_sb[:])
            else:
                # ---- last m-tile: one-hop wide evacuation, host selects —
                # DVE and ACT copy disjoint halves in parallel ----
                o2_sb = opool.tile([128, WCOL], BF16, tag="o2")
                nc.vector.tensor_copy(o2_sb[:, :NCLS], pc_ps[m][:, :NCLS])
                nc.scalar.activation(o2_sb[:, NCLS:], pw_ps[:, :],
                                     mybir.ActivationFunctionType.Copy)
                nc.scalar.dma_start(out2[:, :], o2_sb[:])

    nc.compile()
    return nc


def _route(cls):
    """Group tokens by class into capacity-C slots; tokens beyond capacity
    are returned as `overflow` and evaluated on the host in numpy."""
    counts = np.bincount(cls, minlength=NCLS_PAD)
    order = np.argsort(cls, kind="stable")
    starts = np.zeros(NCLS_PAD + 1, np.int64)
    starts[1:] = np.cumsum(counts)

    tok_idx = np.full((NCORES, NPAD), -1, np.int64)
    overflow = []
    for k in range(NCORES):
        for s in range(SLOTS):
            c = k * CPC + s
            lo, cnt = int(starts[c]), int(counts[c])
            n = min(C, cnt)
            if n > 0:
                tok_idx[k, s * C:s * C + n] = order[lo:lo + n]
            if cnt > C:
                overflow.append(order[lo + C:lo + cnt])
    overflow = (np.concatenate(overflow) if overflow
                else np.zeros((0,), np.int64))
    return tok_idx, overflow


def kernel(x, Wc, bc, Ww, bw, cls_idx, _trace=False, _trace_cores=None):
    global LAST_RESULT
    x = np.ascontiguousarray(np.asarray(x, np.float32))
    Wc = np.ascontiguousarray(np.asarray(Wc, np.float32))
    bc = np.asarray(bc, np.float32)
    Ww = np.ascontiguousarray(np.asarray(Ww, np.float32))
    bw = np.asarray(bw, np.float32)
    cls = np.asarray(cls_idx).astype(np.int64).ravel()
    N = cls.shape[0]

    tok_idx, overflow = _route(cls)

    if "prog" not in _program_cache:
        _program_cache["prog"] = _build_program()
    nc = _program_cache["prog"]

    # wcT [128, KCH*256]: wcT[p, j*256+c] = (Wc*SCL)[c, j*128+p], replicated
    Wc_p = np.zeros((NCLS_PAD, NHID), np.float32)
    Wc_p[:NCLS] = Wc * SCL
    wcT = np.ascontiguousarray(
        Wc_p.reshape(NCLS_PAD, KCH, 128).transpose(2, 1, 0)
            .reshape(128, KCH * NCLS_PAD).astype(NP_BF16))

    # per-partition 16-row parity masks: col0 = 1 on first-class rows of each
    # pair, col1 = complement
    msk_np = np.zeros((128, 8), np.float32)
    msk_np[:, 0] = 1.0 - ((np.arange(128) // C) % 2)
    msk_np[:, 1] = (np.arange(128) // C) % 2

    Ww_pad = np.zeros((NCLS_PAD, CHUNK, NHID), np.float32)
    Ww_pad[:NCLS] = Ww * SCL

    in_maps = []
    for k in range(NCORES):
        # flat pair-major layout:
        # wwT[p, (pair*KCH+j)*400 + a*200 + w] = Ww_pad[cls(slot), w, j*128+p]
        wsel = Ww_pad[np.arange(k * CPC, (k + 1) * CPC)]     # [32,200,512]
        tmp = wsel.reshape(SLOTS, CHUNK, KCH, 128).transpose(0, 2, 3, 1)
        tmp = tmp.reshape(NPAIRS, 2, KCH, 128, CHUNK)
        tmp = tmp.transpose(3, 0, 2, 1, 4)     # [128, pair, j, 2, CHUNK]
        wwT = np.ascontiguousarray(
            tmp.reshape(128, NPAIRS * PCH).astype(NP_FP8))

        ti = tok_idx[k]
        xk = x[np.maximum(ti, 0)] * (1.0 / SCL)
        xk[ti < 0] = 0.0
        xT = np.ascontiguousarray(
            xk.reshape(N_MT, 128, KCH, 128).transpose(3, 0, 2, 1)
              .reshape(128, N_MT * KCH * 128).astype(NP_BF16))
        in_maps.append({"xT": xT, "wcT": wcT, "wwT": wwT, "msk": msk_np})

    LAST_RESULT = run_bass_kernel_spmd(
        nc, in_maps, list(range(NCORES)), trace=_trace,
        trace_cores=(_trace_cores if _trace else None))

    out = np.zeros((N, NCOL), np.float32)
    # last m-tile rows come back wide; slot parity picks the 200-col half
    a_row = ((np.arange(128) // C) % 2)[:, None]
    for k in range(NCORES):
        res = LAST_RESULT.results[k]
        ok = np.asarray(res["out"]).astype(np.float32)
        o2 = np.asarray(res["out2"]).astype(np.float32)
        words2 = np.where(a_row == 0, o2[:, NCLS:NCLS + CHUNK],
                          o2[:, NCLS + CHUNK:])
        full = np.concatenate(
            [ok, np.concatenate([o2[:, :NCLS], words2], 1)], 0)
        valid = tok_idx[k] >= 0
        out[tok_idx[k][valid]] = full[valid]

    if overflow.size:
        xo = x[overflow]
        out[overflow, :NCLS] = xo @ Wc.T
        co = cls[overflow]
        out[overflow, NCLS:] = np.einsum(
            "nkh,nh->nk", Ww[co], xo, optimize=True)

    out[:, :NCLS] += bc
    out[:, NCLS:] += bw[cls]
    return out
